# revision 1
# baseline (speedup 1.0000x reference)
"""Trainium2 Bass kernel for the masked per-site stencil contraction

    y[o, n] = f( sum_{i,k} Wconv[o,i,k] * mask[n,o,i,k] * x[i, shifts[n,k]] + bconv[o] )
    f(v) = (sigmoid(v) - 0.5) * (2 + 2e)/(e - 1) = (2+2e)/(2(e-1)) * tanh(v/2)

Shapes: O=I=32, K=13, N=4096.  Sharded over 8 NeuronCores along the site
dimension N (512 sites per core); mask / shifts / output columns are
partitioned, x / Wconv / bconv replicated.

Per-core device plan (all cores run the identical SPMD program):
  * layout: partition dim = (k, i) rows of the 416-long stencil axis
    (k-major, p = k*32 + i), free dim = local sites n (512).
    Chunks c=0..2 cover k in [4c, 4c+4) -> 128 partitions each; the k=12
    remainder is packed 4-output-channels-per-128-partition tile, with
    zero-padded weight columns selecting each channel's 32 rows.
  * gather g[p, n] = x[i(p), shifts[n, k(p)]] with GPSIMD ap_gather
    (x replicated to 128 partitions; indices pre-wrapped host-side).
  * DVE: prod = mask_tile * g  (the only full-size elementwise pass)
  * PE:  y[o, n] = sum_p W[o, p] * prod_o[p, n] as a 4-chunk accumulated
    matvec per output channel, lhsT = W column, float32r (1 cyc/row).
  * ACT: y = tanh(0.5*y + 0.5*b) per channel from PSUM; batched staging
    DMA; DVE: * scale/2; DMA out.
  * mask DMAs alternate between the two HWDGE rings (sync / scalar).
"""

import math

import numpy as np

import concourse.bacc as bacc
import concourse.mybir as mybir
from concourse import tile
from concourse.bass_utils import run_bass_kernel_spmd

O, I, K, N = 32, 32, 13, 4096
NCORES = 8
NS = N // NCORES          # 512 local sites per core
IK = K * I                # 416 stencil rows, k-major: p = k*32 + i
BIG = 12 * I              # 384 rows in the three 128-partition chunks
GROUP_ROWS = 4 * BIG + 128  # 1664 mask rows per 4-channel group
_E = math.e
SCALE = (2.0 + 2.0 * _E) / (_E - 1.0)

_F32 = mybir.dt.float32
_F32R = mybir.dt.float32r
_BF16 = mybir.dt.float16
_I16 = mybir.dt.int16

_BUILT = {}


def _emit(nc, tc, d, pools):
    """Emit one full per-core computation (used by kernel + timing builds)."""
    cpool, gpool, mpool, m3pool, ppool, p3pool, opool, qpool = pools

    x_sb = cpool.tile([128, N], _F32, tag="x")
    nc.sync.dma_start(x_sb[:, :], d["x4"][:, :])
    wt_big = cpool.tile([128, 3, O], _F32, tag="wb")
    nc.sync.dma_start(
        wt_big[:, :, :], d["wt"][0:BIG, :].rearrange("(c p) m -> p c m", p=128)
    )
    wt3f = cpool.tile([128, 4, O], _F32, tag="w3")
    nc.sync.dma_start(wt3f[:, :, :], d["wt3f"][:, :, :].rearrange("j p m -> p j m"))
    wt_bigr = cpool.tile([128, 3, O], _BF16, tag="wbr")
    nc.vector.tensor_copy(wt_bigr[:, :, :], wt_big[:, :, :])
    wt3fr = cpool.tile([128, 4, O], _BF16, tag="w3r")
    nc.vector.tensor_copy(wt3fr[:, :, :], wt3f[:, :, :])
    idxb_sb = cpool.tile([128, 96], _I16, tag="ib")
    nc.sync.dma_start(idxb_sb[:, :], d["idxb"][:, :])
    idx3_sb = cpool.tile([128, 32], _I16, tag="i3")
    nc.sync.dma_start(idx3_sb[:, :], d["idx3"][:, :])
    brow_sb = cpool.tile([1, O], _F32, tag="br")
    nc.sync.dma_start(brow_sb[:, :], d["brow"][:, :])

    # g3rep[p, n] = x[p%32, shifts[n, 12]]  (k=12 row, replicated x4).
    # Issued first: it unblocks the per-group p3p products early while the
    # per-chunk gathers below stream in.
    g3rep = gpool.tile([128, NS], _F32, tag="g3")
    nc.gpsimd.ap_gather(
        g3rep[:, :], x_sb[:, :], idx3_sb[:, :],
        channels=128, num_elems=N, d=1, num_idxs=NS,
    )
    g3b = gpool.tile([128, NS], _BF16, tag="g3b")
    nc.vector.tensor_copy(g3b[:, :], g3rep[:, :])
    # g[p, c*512 + n] = x[p%32, shifts[n, 4c + p//32]], one call per chunk c
    # so chunk-c compute can start before later chunks are gathered.
    g_big = gpool.tile([128, 3, NS], _F32, tag="g")
    gb = gpool.tile([128, 3, NS], _BF16, tag="gb")
    for c in range(3):
        nc.gpsimd.ap_gather(
            g_big[:, c, :], x_sb[:, :], idxb_sb[:, 32 * c : 32 * c + 32],
            channels=128, num_elems=N, d=1, num_idxs=NS,
        )
        nc.vector.tensor_copy(gb[:, c, :], g_big[:, c, :])

    bhalf = opool.tile([1, O], _F32, tag="bh")
    nc.scalar.activation(
        bhalf[:, :], brow_sb[:, :], mybir.ActivationFunctionType.Copy, scale=0.5
    )
    ystage = opool.tile([O, NS], _F32, tag="ys")

    d_m = d["maskg"]

    # --- k=12 products first: they only need g3rep (gathered first), filling
    # the DVE while the per-chunk gathers stream in.
    p3ps = []
    for og in range(O // 4):
        m3p = m3pool.tile([128, NS], _F32, tag="m3p")
        nc.scalar.dma_start(m3p[:, :], d_m[og, 4 * BIG : GROUP_ROWS, :])
        m3b = m3pool.tile([128, NS], _BF16, tag="m3b")
        nc.scalar.copy(m3b[:, :], m3p[:, :])
        p3p = p3pool.tile([128, NS], _BF16, tag="p3p", bufs=8)
        nc.vector.tensor_mul(p3p[:, :], m3b[:, :], g3b[:, :])
        p3ps.append(p3p)

    # --- wave phase: channels 0..NW-1 processed chunk-major so each chunk's
    # products start as soon as that chunk's gather lands; products are
    # buffered until the per-channel PE accumulation below.
    NW = 12
    pts = {}
    for c in range(2):
        for o in range(NW):
            og, j = divmod(o, 4)
            mtc = mpool.tile([128, NS], _F32, tag="mtc")
            eng = nc.sync if o % 2 == 0 else nc.scalar
            eng.dma_start(
                mtc[:, :],
                d_m[og, j * BIG + c * 128 : j * BIG + (c + 1) * 128, :],
            )
            mtb = mpool.tile([128, NS], _BF16, tag="mtb")
            nc.scalar.copy(mtb[:, :], mtc[:, :])
            ptc = ppool.tile([128, NS], _BF16, tag=f"pt{c}_{o}", bufs=1)
            nc.vector.tensor_mul(ptc[:, :], mtb[:, :], gb[:, c, :])
            pts[(c, o)] = ptc

    def chan_tail(o, yp, ycat):
        og, j = divmod(o, 4)
        nc.scalar.activation(
            ycat[0:1, j, :], yp[:, :], mybir.ActivationFunctionType.Tanh,
            bias=bhalf[0:1, o : o + 1], scale=0.5,
        )
        if j == 3:
            nc.sync.dma_start(
                ystage[4 * og : 4 * og + 4, :], ycat[0:1, :, :]
            )

    ycat = None
    for o in range(NW):
        og, j = divmod(o, 4)
        if j == 0:
            ycat = opool.tile([1, 4, NS], _F32, tag="ycat", bufs=2)
        mtc = mpool.tile([128, NS], _F32, tag="mtc")
        eng = nc.sync if o % 2 == 0 else nc.scalar
        eng.dma_start(
            mtc[:, :], d_m[og, j * BIG + 2 * 128 : j * BIG + 3 * 128, :]
        )
        mtb = mpool.tile([128, NS], _BF16, tag="mtb")
        nc.scalar.copy(mtb[:, :], mtc[:, :])
        ptc = ppool.tile([128, NS], _BF16, tag="ptc2", bufs=4)
        nc.vector.tensor_mul(ptc[:, :], mtb[:, :], gb[:, 2, :])
        yp = qpool.tile([1, NS], _F32, tag="yp", bufs=4)
        nc.tensor.matmul(
            yp[:, :], wt3fr[:, j, o : o + 1], p3ps[og][:, :],
            start=True, stop=False,
        )
        for c in range(2):
            nc.tensor.matmul(
                yp[:, :], wt_bigr[:, c, o : o + 1], pts.pop((c, o))[:, :],
                start=False, stop=False,
            )
        nc.tensor.matmul(
            yp[:, :], wt_bigr[:, 2, o : o + 1], ptc[:, :],
            start=False, stop=True,
        )
        chan_tail(o, yp, ycat)

    # --- remaining channels: all gathers done by now, plain channel-major
    for o in range(NW, O):
        og, j = divmod(o, 4)
        if j == 0:
            ycat = opool.tile([1, 4, NS], _F32, tag="ycat", bufs=2)
        mt = mpool.tile([128, 3, NS], _F32, tag="mt")
        eng = nc.sync if o % 2 == 0 else nc.scalar
        eng.dma_start(
            mt[:, :, :],
            d_m[og, j * BIG : (j + 1) * BIG, :].rearrange("(c p) n -> p c n", p=128),
        )
        mtb3 = mpool.tile([128, 3, NS], _BF16, tag="mtb3")
        nc.scalar.copy(mtb3[:, :, :], mt[:, :, :])
        pt = ppool.tile([128, 3, NS], _BF16, tag="pt")
        yp = qpool.tile([1, NS], _F32, tag="yp", bufs=4)
        nc.tensor.matmul(
            yp[:, :], wt3fr[:, j, o : o + 1], p3ps[og][:, :],
            start=True, stop=False,
        )
        for c in range(3):
            nc.vector.tensor_mul(pt[:, c, :], mtb3[:, c, :], gb[:, c, :])
            nc.tensor.matmul(
                yp[:, :], wt_bigr[:, c, o : o + 1], pt[:, c, :],
                start=False, stop=(c == 2),
            )
        chan_tail(o, yp, ycat)

    nc.vector.tensor_scalar_mul(ystage[:, :], ystage[:, :], SCALE / 2.0)
    nc.sync.dma_start(d["y"][:, :], ystage[:, :])


def _declare(nc):
    d = {}
    d["x4"] = nc.declare_dram_parameter("x4", [128, N], _F32, isOutput=False)
    d["wt"] = nc.declare_dram_parameter("wt", [IK, O], _F32, isOutput=False)
    d["wt3f"] = nc.declare_dram_parameter("wt3f", [4, 128, O], _F32, isOutput=False)
    d["brow"] = nc.declare_dram_parameter("brow", [1, O], _F32, isOutput=False)
    d["maskg"] = nc.declare_dram_parameter(
        "maskg", [O // 4, GROUP_ROWS, NS], _F32, isOutput=False
    )
    d["idxb"] = nc.declare_dram_parameter("idxb", [128, 96], _I16, isOutput=False)
    d["idx3"] = nc.declare_dram_parameter("idx3", [128, 32], _I16, isOutput=False)
    d["y"] = nc.declare_dram_parameter("y", [O, NS], _F32, isOutput=True)
    return d


def _pools(tc, stack):
    names = [
        ("const", 1), ("gather", 1), ("mask", 4), ("m3", 3),
        ("prod", 3), ("p3", 3), ("out", 1), ("psum", 1),
    ]
    pools = []
    for name, bufs in names:
        kw = {"space": "PSUM"} if name == "psum" else {}
        pools.append(stack.enter_context(tc.tile_pool(name=name, bufs=bufs, **kw)))
    return pools


def _build():
    """Build + compile the SPMD Bass program once per process."""
    if "nc" in _BUILT:
        return _BUILT["nc"]
    from contextlib import ExitStack

    nc = bacc.Bacc("TRN2", target_bir_lowering=False, debug=False)
    d = _declare(nc)
    with tile.TileContext(nc) as tc:
        with ExitStack() as stack:
            pools = _pools(tc, stack)
            _emit(nc, tc, d, pools)
    nc.compile()
    _BUILT["nc"] = nc
    return nc


def _wrap16(col):
    """shifts column (NS,) -> (16, NS//16) wrapped layout: out[r, s] = col[s*16+r]."""
    return np.ascontiguousarray(col.reshape(NS // 16, 16).T)


def make_in_maps(x, Wconv, bconv, mask, shifts):
    """Host-side shard/layout prep. Pure data movement (+ dtype-preserving
    int32->int16 index narrowing; indices are < 4096)."""
    x = np.ascontiguousarray(x, dtype=np.float32)
    x4 = np.ascontiguousarray(np.tile(x, (4, 1)))                   # (128, N)
    W = Wconv.astype(np.float32, copy=False)
    wt = np.ascontiguousarray(W.transpose(2, 1, 0)).reshape(IK, O)  # (416, 32)
    # zero-padded k=12 weight columns: wt3f[j, 32j+i, o] = W[o, i, 12]
    wt3f = np.zeros((4, 128, O), np.float32)
    for j in range(4):
        wt3f[j, 32 * j : 32 * j + 32, :] = W[:, :, 12].T
    brow = np.ascontiguousarray(bconv.astype(np.float32, copy=False).reshape(1, O))
    mask = np.asarray(mask, dtype=np.float32)
    shifts = np.asarray(shifts)

    in_maps = []
    for core in range(NCORES):
        sl = slice(core * NS, (core + 1) * NS)
        mt = np.ascontiguousarray(mask[sl].transpose(1, 3, 2, 0))   # (O, K, I, NS)
        big = mt[:, :12].reshape(O // 4, 4 * BIG, NS)
        k12 = mt[:, 12].reshape(O // 4, 128, NS)
        maskg = np.ascontiguousarray(
            np.concatenate([big, k12], axis=1)
        )                                                           # (8, 1664, NS)
        sh = shifts[sl].astype(np.int16)                            # (NS, 13)
        idxb = np.empty((128, 96), np.int16)
        for g in range(8):
            for c in range(3):
                idxb[16 * g : 16 * g + 16, 32 * c : 32 * c + 32] = _wrap16(
                    sh[:, 4 * c + g // 2]
                )
        w12 = _wrap16(sh[:, 12])
        idx3 = np.empty((128, 32), np.int16)
        for g in range(8):
            idx3[16 * g : 16 * g + 16, :] = w12
        in_maps.append(
            {
                "x4": x4,
                "wt": wt,
                "wt3f": wt3f,
                "brow": brow,
                "maskg": maskg,
                "idxb": idxb,
                "idx3": idx3,
            }
        )
    return in_maps


def kernel(x, Wconv, bconv, mask, shifts):
    nc = _build()
    in_maps = make_in_maps(x, Wconv, bconv, mask, shifts)
    res = run_bass_kernel_spmd(nc, in_maps, core_ids=list(range(NCORES)))
    y = np.empty((O, N), np.float32)
    for core in range(NCORES):
        y[:, core * NS : (core + 1) * NS] = res.results[core]["y"]
    return y



# revision 7
# speedup vs baseline: 1.1849x; 1.1849x over previous
"""Trainium2 Bass kernel for the masked per-site stencil contraction

    y[o, n] = f( sum_{i,k} Wconv[o,i,k] * mask[n,o,i,k] * x[i, shifts[n,k]] + bconv[o] )
    f(v) = (sigmoid(v) - 0.5) * (2 + 2e)/(e - 1) = (2+2e)/(2(e-1)) * tanh(v/2)

Shapes: O=I=32, K=13, N=4096.  Sharded over 8 NeuronCores along the site
dimension N (512 sites per core); mask / shifts / output columns are
partitioned, x / Wconv / bconv replicated.

Per-core device plan (all cores run the identical SPMD program):
  * layout: partition dim = (k, i) rows of the 416-long stencil axis
    (k-major, p = k*32 + i), free dim = local sites n (512).
    Chunks c=0..2 cover k in [4c, 4c+4) -> 128 partitions each; the k=12
    remainder is packed 4-output-channels-per-128-partition tile, with
    zero-padded weight columns selecting each channel's 32 rows.
  * gather g[p, n] = x[i(p), shifts[n, k(p)]] with GPSIMD ap_gather
    (x replicated to 128 partitions; indices pre-wrapped host-side).
  * DVE: prod = mask_tile * g  (the only full-size elementwise pass)
  * PE:  y[o, n] = sum_p W[o, p] * prod_o[p, n] as a 4-chunk accumulated
    matvec per output channel, lhsT = W column, float32r (1 cyc/row).
  * ACT: y = tanh(0.5*y + 0.5*b) per channel from PSUM; batched staging
    DMA; DVE: * scale/2; DMA out.
  * mask DMAs alternate between the two HWDGE rings (sync / scalar).
"""

import math

import numpy as np

import concourse.bacc as bacc
import concourse.mybir as mybir
from concourse import tile
from concourse.bass_utils import run_bass_kernel_spmd

O, I, K, N = 32, 32, 13, 4096
NCORES = 8
NS = N // NCORES          # 512 local sites per core
IK = K * I                # 416 stencil rows, k-major: p = k*32 + i
BIG = 12 * I              # 384 rows in the three 128-partition chunks
GROUP_ROWS = 4 * BIG + 128  # 1664 mask rows per 4-channel group
_E = math.e
SCALE = (2.0 + 2.0 * _E) / (_E - 1.0)

_F32 = mybir.dt.float32
_F32R = mybir.dt.float32r
_BF16 = mybir.dt.float16
_I16 = mybir.dt.int16

_BUILT = {}


def _emit(nc, tc, d, pools):
    """Emit one full per-core computation (used by kernel + timing builds)."""
    cpool, gpool, mpool, m3pool, ppool, p3pool, opool, qpool = pools

    x_sb = cpool.tile([128, N], _F32, tag="x")
    nc.sync.dma_start(x_sb[:, :], d["x4"][:, :])
    wt_big = cpool.tile([128, 3, O], _F32, tag="wb")
    nc.sync.dma_start(
        wt_big[:, :, :], d["wt"][0:BIG, :].rearrange("(c p) m -> p c m", p=128)
    )
    wt3f = cpool.tile([128, 4, O], _F32, tag="w3")
    nc.sync.dma_start(wt3f[:, :, :], d["wt3f"][:, :, :].rearrange("j p m -> p j m"))
    wt_bigr = cpool.tile([128, 3, O], _BF16, tag="wbr")
    nc.vector.tensor_copy(wt_bigr[:, :, :], wt_big[:, :, :])
    wt3fr = cpool.tile([128, 4, O], _BF16, tag="w3r")
    nc.vector.tensor_copy(wt3fr[:, :, :], wt3f[:, :, :])
    idxb_sb = cpool.tile([128, 96], _I16, tag="ib")
    nc.sync.dma_start(idxb_sb[:, :], d["idxb"][:, :])
    idx3_sb = cpool.tile([128, 32], _I16, tag="i3")
    nc.sync.dma_start(idx3_sb[:, :], d["idx3"][:, :])
    brow_sb = cpool.tile([1, O], _F32, tag="br")
    nc.sync.dma_start(brow_sb[:, :], d["brow"][:, :])

    # g3rep[p, n] = x[p%32, shifts[n, 12]]  (k=12 row, replicated x4).
    # Issued first: it unblocks the per-group p3p products early while the
    # per-chunk gathers below stream in.
    g3rep = gpool.tile([128, NS], _F32, tag="g3")
    nc.gpsimd.ap_gather(
        g3rep[:, :], x_sb[:, :], idx3_sb[:, :],
        channels=128, num_elems=N, d=1, num_idxs=NS,
    )
    g3b = gpool.tile([128, NS], _BF16, tag="g3b")
    nc.vector.tensor_copy(g3b[:, :], g3rep[:, :])
    # g[p, c*512 + n] = x[p%32, shifts[n, 4c + p//32]], one call per chunk c
    # so chunk-c compute can start before later chunks are gathered.
    g_big = gpool.tile([128, 3, NS], _F32, tag="g")
    gb = gpool.tile([128, 3, NS], _BF16, tag="gb")
    for c in range(3):
        nc.gpsimd.ap_gather(
            g_big[:, c, :], x_sb[:, :], idxb_sb[:, 32 * c : 32 * c + 32],
            channels=128, num_elems=N, d=1, num_idxs=NS,
        )
        nc.vector.tensor_copy(gb[:, c, :], g_big[:, c, :])

    bhalf = opool.tile([1, O], _F32, tag="bh")
    nc.scalar.activation(
        bhalf[:, :], brow_sb[:, :], mybir.ActivationFunctionType.Copy, scale=0.5
    )
    ystage = opool.tile([O, NS], _F32, tag="ys")

    d_m = d["maskg"]

    # --- k=12 products first: they only need g3rep (gathered first), filling
    # the DVE while the per-chunk gathers stream in.
    p3ps = []
    for og in range(O // 4):
        m3p = m3pool.tile([128, NS], _BF16, tag="m3p")
        nc.scalar.dma_start(m3p[:, :], d_m[og, 4 * BIG : GROUP_ROWS, :])
        p3p = p3pool.tile([128, NS], _BF16, tag="p3p", bufs=8)
        nc.vector.tensor_mul(p3p[:, :], m3p[:, :], g3b[:, :])
        p3ps.append(p3p)

    # --- wave phase: channels 0..NW-1 processed chunk-major so each chunk's
    # products start as soon as that chunk's gather lands; products are
    # buffered until the per-channel PE accumulation below.
    NW = 12
    pts = {}
    for c in range(2):
        for o in range(NW):
            og, j = divmod(o, 4)
            mtc = mpool.tile([128, NS], _BF16, tag="mtc")
            eng = nc.sync if o % 2 == 0 else nc.scalar
            eng.dma_start(
                mtc[:, :],
                d_m[og, j * BIG + c * 128 : j * BIG + (c + 1) * 128, :],
            )
            ptc = ppool.tile([128, NS], _BF16, tag=f"pt{c}_{o}", bufs=1)
            nc.vector.tensor_mul(ptc[:, :], mtc[:, :], gb[:, c, :])
            pts[(c, o)] = ptc

    def chan_tail(o, yp, ycat):
        og, j = divmod(o, 4)
        nc.scalar.activation(
            ycat[0:1, j, :], yp[:, :], mybir.ActivationFunctionType.Tanh,
            bias=bhalf[0:1, o : o + 1], scale=0.5,
        )
        if j == 3:
            nc.sync.dma_start(
                ystage[4 * og : 4 * og + 4, :], ycat[0:1, :, :]
            )

    ycat = None
    for o in range(NW):
        og, j = divmod(o, 4)
        if j == 0:
            ycat = opool.tile([1, 4, NS], _F32, tag="ycat", bufs=2)
        mtc = mpool.tile([128, NS], _BF16, tag="mtc")
        eng = nc.sync if o % 2 == 0 else nc.scalar
        eng.dma_start(
            mtc[:, :], d_m[og, j * BIG + 2 * 128 : j * BIG + 3 * 128, :]
        )
        ptc = ppool.tile([128, NS], _BF16, tag="ptc2", bufs=4)
        nc.vector.tensor_mul(ptc[:, :], mtc[:, :], gb[:, 2, :])
        yp = qpool.tile([1, NS], _F32, tag="yp", bufs=4)
        nc.tensor.matmul(
            yp[:, :], wt3fr[:, j, o : o + 1], p3ps[og][:, :],
            start=True, stop=False,
        )
        for c in range(2):
            nc.tensor.matmul(
                yp[:, :], wt_bigr[:, c, o : o + 1], pts.pop((c, o))[:, :],
                start=False, stop=False,
            )
        nc.tensor.matmul(
            yp[:, :], wt_bigr[:, 2, o : o + 1], ptc[:, :],
            start=False, stop=True,
        )
        chan_tail(o, yp, ycat)

    # --- remaining channels: all gathers done by now, plain channel-major
    for o in range(NW, O):
        og, j = divmod(o, 4)
        if j == 0:
            ycat = opool.tile([1, 4, NS], _F32, tag="ycat", bufs=2)
        mt = mpool.tile([128, 3, NS], _BF16, tag="mt")
        eng = nc.sync if o % 2 == 0 else nc.scalar
        eng.dma_start(
            mt[:, :, :],
            d_m[og, j * BIG : (j + 1) * BIG, :].rearrange("(c p) n -> p c n", p=128),
        )
        pt = ppool.tile([128, 3, NS], _BF16, tag="pt")
        yp = qpool.tile([1, NS], _F32, tag="yp", bufs=4)
        nc.tensor.matmul(
            yp[:, :], wt3fr[:, j, o : o + 1], p3ps[og][:, :],
            start=True, stop=False,
        )
        for c in range(3):
            nc.vector.tensor_mul(pt[:, c, :], mt[:, c, :], gb[:, c, :])
            nc.tensor.matmul(
                yp[:, :], wt_bigr[:, c, o : o + 1], pt[:, c, :],
                start=False, stop=(c == 2),
            )
        chan_tail(o, yp, ycat)

    nc.vector.tensor_scalar_mul(ystage[:, :], ystage[:, :], SCALE / 2.0)
    nc.sync.dma_start(d["y"][:, :], ystage[:, :])


def _declare(nc):
    d = {}
    d["x4"] = nc.declare_dram_parameter("x4", [128, N], _F32, isOutput=False)
    d["wt"] = nc.declare_dram_parameter("wt", [IK, O], _F32, isOutput=False)
    d["wt3f"] = nc.declare_dram_parameter("wt3f", [4, 128, O], _F32, isOutput=False)
    d["brow"] = nc.declare_dram_parameter("brow", [1, O], _F32, isOutput=False)
    d["maskg"] = nc.declare_dram_parameter(
        "maskg", [O // 4, GROUP_ROWS, NS], _BF16, isOutput=False
    )
    d["idxb"] = nc.declare_dram_parameter("idxb", [128, 96], _I16, isOutput=False)
    d["idx3"] = nc.declare_dram_parameter("idx3", [128, 32], _I16, isOutput=False)
    d["y"] = nc.declare_dram_parameter("y", [O, NS], _F32, isOutput=True)
    return d


def _pools(tc, stack):
    names = [
        ("const", 1), ("gather", 1), ("mask", 4), ("m3", 3),
        ("prod", 3), ("p3", 3), ("out", 1), ("psum", 1),
    ]
    pools = []
    for name, bufs in names:
        kw = {"space": "PSUM"} if name == "psum" else {}
        pools.append(stack.enter_context(tc.tile_pool(name=name, bufs=bufs, **kw)))
    return pools


def _build():
    """Build + compile the SPMD Bass program once per process."""
    if "nc" in _BUILT:
        return _BUILT["nc"]
    from contextlib import ExitStack

    nc = bacc.Bacc("TRN2", target_bir_lowering=False, debug=False)
    d = _declare(nc)
    with tile.TileContext(nc) as tc:
        with ExitStack() as stack:
            pools = _pools(tc, stack)
            _emit(nc, tc, d, pools)
    nc.compile()
    _BUILT["nc"] = nc
    return nc


def _wrap16(col):
    """shifts column (NS,) -> (16, NS//16) wrapped layout: out[r, s] = col[s*16+r]."""
    return np.ascontiguousarray(col.reshape(NS // 16, 16).T)


def make_in_maps(x, Wconv, bconv, mask, shifts):
    """Host-side shard/layout prep. Pure data movement (+ dtype-preserving
    int32->int16 index narrowing; indices are < 4096)."""
    x = np.ascontiguousarray(x, dtype=np.float32)
    x4 = np.ascontiguousarray(np.tile(x, (4, 1)))                   # (128, N)
    W = Wconv.astype(np.float32, copy=False)
    wt = np.ascontiguousarray(W.transpose(2, 1, 0)).reshape(IK, O)  # (416, 32)
    # zero-padded k=12 weight columns: wt3f[j, 32j+i, o] = W[o, i, 12]
    wt3f = np.zeros((4, 128, O), np.float32)
    for j in range(4):
        wt3f[j, 32 * j : 32 * j + 32, :] = W[:, :, 12].T
    brow = np.ascontiguousarray(bconv.astype(np.float32, copy=False).reshape(1, O))
    mask = np.asarray(mask, dtype=np.float32)
    shifts = np.asarray(shifts)

    in_maps = []
    for core in range(NCORES):
        sl = slice(core * NS, (core + 1) * NS)
        mt = np.ascontiguousarray(
            mask[sl].transpose(1, 3, 2, 0).astype(np.float16)
        )                                                           # (O, K, I, NS)
        big = mt[:, :12].reshape(O // 4, 4 * BIG, NS)
        k12 = mt[:, 12].reshape(O // 4, 128, NS)
        maskg = np.ascontiguousarray(
            np.concatenate([big, k12], axis=1)
        )                                                           # (8, 1664, NS)
        sh = shifts[sl].astype(np.int16)                            # (NS, 13)
        idxb = np.empty((128, 96), np.int16)
        for g in range(8):
            for c in range(3):
                idxb[16 * g : 16 * g + 16, 32 * c : 32 * c + 32] = _wrap16(
                    sh[:, 4 * c + g // 2]
                )
        w12 = _wrap16(sh[:, 12])
        idx3 = np.empty((128, 32), np.int16)
        for g in range(8):
            idx3[16 * g : 16 * g + 16, :] = w12
        in_maps.append(
            {
                "x4": x4,
                "wt": wt,
                "wt3f": wt3f,
                "brow": brow,
                "maskg": maskg,
                "idxb": idxb,
                "idx3": idx3,
            }
        )
    return in_maps


def kernel(x, Wconv, bconv, mask, shifts):
    nc = _build()
    in_maps = make_in_maps(x, Wconv, bconv, mask, shifts)
    res = run_bass_kernel_spmd(nc, in_maps, core_ids=list(range(NCORES)))
    y = np.empty((O, N), np.float32)
    for core in range(NCORES):
        y[:, core * NS : (core + 1) * NS] = res.results[core]["y"]
    return y



# revision 13
# speedup vs baseline: 1.9337x; 1.6320x over previous
"""Trainium2 Bass kernel for the masked per-site stencil contraction

    y[o, n] = f( sum_{i,k} Wconv[o,i,k] * mask[n,o,i,k] * x[i, shifts[n,k]] + bconv[o] )
    f(v) = (sigmoid(v) - 0.5) * (2 + 2e)/(e - 1) = (2+2e)/(2(e-1)) * tanh(v/2)

Shapes: O=I=32, K=13, N=4096.  Sharded over 8 NeuronCores along the site
dimension N (512 sites per core); mask / shifts / output columns are
partitioned, x / Wconv / bconv replicated.

Per-core device plan (all cores run the identical SPMD program):
  * g built by 13 dma_gather calls (one per tap k) from xT4 in HBM, where
    xT4[s, 32a+i] = x[i, s] (x^T replicated 4x along features, 256B rows).
    transpose=True lands the feature dim on partitions: g_k[32a+i, n] =
    x[i, shifts[n, k]].  SWDGE descriptors prep on GPSIMD, data moves on
    the 16 DMA engines across 4 SWDGE queues.
  * mask shipped as fp16 (exact for a 0/1 mask) in [og, k, (j,i), n]
    layout: 4 output channels j packed along the 128-partition dim.
  * DVE: prod[(j,i), n] = mask_tile[og, k] * g_k  (fp16, 2x_1P mode)
  * PE:  one m=4 matmul per (og, k): ypsum[4og:4og+4, n] += W4^T @ prod,
    13-long accumulation chains per og, all 8 chains in ONE PSUM bank
    ([32, 512] f32).  (og, k) emission follows a diagonal sort matching
    DMA/gather arrival order.
  * ACT: single tanh over [32, 512] PSUM with per-partition bias; DVE
    scale; one output DMA.
"""

import math

import numpy as np

import concourse.bacc as bacc
import concourse.mybir as mybir
from concourse import tile
from concourse.bass_utils import run_bass_kernel_spmd

O, I, K, N = 32, 32, 13, 4096
NCORES = 8
NS = N // NCORES          # 512 local sites per core
NOG = O // 4              # 8 channel groups of 4
_E = math.e
SCALE = (2.0 + 2.0 * _E) / (_E - 1.0)

_F32 = mybir.dt.float32
_F16 = mybir.dt.float16
_I16 = mybir.dt.int16

_BUILT = {}


def _emit(nc, tc, d, pools):
    cpool, gpool, mpool, ppool, opool, qpool = pools

    idx_sb = cpool.tile([128, K * 32], _I16, tag="idx")
    nc.sync.dma_start(idx_sb[:, :], d["idx"][:, :])
    wt4f = cpool.tile([128, K, NOG, 4], _F32, tag="w4f")
    nc.scalar.dma_start(wt4f[:, :, :, :], d["wt4"][:, :, :, :])
    bcol_sb = cpool.tile([4, NOG], _F32, tag="bc")
    nc.scalar.dma_start(bcol_sb[:, :], d["bcol"][:, :])

    # g[32a+i, k, 0, n] = x[i, shifts[n, k]] via SWDGE gather from xT4 rows
    g = gpool.tile([128, K, 1, NS], _F16, tag="g")
    for k in range(K):
        nc.gpsimd.dma_gather(
            g[:, k, :, :],
            d["xT4"][:, :],
            idx_sb[:, 32 * k : 32 * (k + 1)],
            num_idxs=NS,
            num_idxs_reg=NS,
            elem_size=128,
            transpose=True,
            queue_num=k % 4,
        )

    # all 8 og mask tiles stay resident (13.3 KB/partition each)
    mts = []
    for og in range(NOG):
        mt = mpool.tile([128, K, NS], _F16, tag=f"m{og}", bufs=1)
        eng = nc.sync if og % 2 == 0 else nc.scalar
        eng.dma_start(
            mt[:, :, :], d["maskg"][og].rearrange("k p n -> p k n")
        )
        mts.append(mt)

    wt4 = cpool.tile([128, K, NOG, 4], _F16, tag="w4")
    nc.vector.tensor_copy(wt4[:, :, :, :], wt4f[:, :, :, :])
    bhalf = opool.tile([4, NOG], _F32, tag="bh")
    nc.scalar.activation(
        bhalf[:, :], bcol_sb[:, :], mybir.ActivationFunctionType.Copy, scale=0.5
    )

    # one PSUM bank per og chain: [4, 512] f32 at base partition 0
    yps = [
        qpool.tile([4, NS], _F32, tag=f"yp{og}", bufs=1, name=f"yp{og}")
        for og in range(NOG)
    ]

    # diagonal emission: (og, k) sorted by expected arrival of the later of
    # (gather k, mask og); keeps both DVE and PE fed from the first arrivals.
    order = sorted(
        ((og, k) for og in range(NOG) for k in range(K)),
        key=lambda t: (max(1.55 * t[1], 2.4 * (t[0] // 2)), t[1], t[0]),
    )
    for og, k in order:
        prod = ppool.tile([128, NS], _F16, tag="prod", bufs=6)
        nc.vector.tensor_mul(prod[:, :], mts[og][:, k, :], g[:, k, 0, :])
        nc.tensor.matmul(
            yps[og][:, :],
            wt4[:, k, og, :],
            prod[:, :],
            start=(k == 0),
            stop=(k == K - 1),
        )
        if k == K - 1:
            ycat = opool.tile([4, NS], _F32, tag="ycat", bufs=2)
            nc.scalar.activation(
                ycat[:, :], yps[og][:, :], mybir.ActivationFunctionType.Tanh,
                bias=bhalf[:, og : og + 1], scale=0.5,
            )
            nc.vector.tensor_scalar_mul(ycat[:, :], ycat[:, :], SCALE / 2.0)
            eng = nc.sync if og % 2 == 0 else nc.scalar
            eng.dma_start(d["y"][4 * og : 4 * og + 4, :], ycat[:, :])


def _declare(nc):
    d = {}
    d["xT4"] = nc.declare_dram_parameter("xT4", [N, 128], _F16, isOutput=False)
    d["idx"] = nc.declare_dram_parameter("idx", [128, K * 32], _I16, isOutput=False)
    d["maskg"] = nc.declare_dram_parameter(
        "maskg", [NOG, K, 128, NS], _F16, isOutput=False
    )
    d["wt4"] = nc.declare_dram_parameter("wt4", [128, K, NOG, 4], _F32, isOutput=False)
    d["bcol"] = nc.declare_dram_parameter("bcol", [4, NOG], _F32, isOutput=False)
    d["y"] = nc.declare_dram_parameter("y", [O, NS], _F32, isOutput=True)
    return d


def _pools(tc, stack):
    names = [
        ("const", 1), ("gather", 1), ("mask", 8), ("prod", 6),
        ("out", 1), ("psum", 1),
    ]
    pools = []
    for name, bufs in names:
        kw = {"space": "PSUM"} if name == "psum" else {}
        pools.append(stack.enter_context(tc.tile_pool(name=name, bufs=bufs, **kw)))
    return pools


def _build():
    if "nc" in _BUILT:
        return _BUILT["nc"]
    from contextlib import ExitStack

    nc = bacc.Bacc(
        "TRN2", target_bir_lowering=False, debug=False, num_swdge_queues=4
    )
    d = _declare(nc)
    with tile.TileContext(nc) as tc:
        with ExitStack() as stack:
            pools = _pools(tc, stack)
            _emit(nc, tc, d, pools)
    nc.compile()
    _BUILT["nc"] = nc
    return nc


def _wrap16(col):
    """shifts column (NS,) -> (16, NS//16) wrapped layout: out[r, s] = col[s*16+r]."""
    return np.ascontiguousarray(col.reshape(NS // 16, 16).T)


def make_in_maps(x, Wconv, bconv, mask, shifts):
    """Host-side shard/layout prep: transposes/reshapes plus value-preserving
    dtype narrowing (f32 0/1 mask -> fp16 exactly; int32 indices < 4096 ->
    int16)."""
    x = np.asarray(x, dtype=np.float32)
    xT4 = np.ascontiguousarray(np.tile(x.T.astype(np.float16), (1, 4)))  # (N, 128)
    W = np.asarray(Wconv, dtype=np.float32)
    W4d = W.reshape(NOG, 4, I, K)                                # (og, j, i, k)
    wt4 = np.zeros((128, K, NOG, 4), np.float32)
    for j in range(4):
        wt4[32 * j : 32 * (j + 1), :, :, j] = W4d[:, j].transpose(1, 2, 0)
    bcol = np.ascontiguousarray(
        np.asarray(bconv, dtype=np.float32).reshape(NOG, 4).T
    )                                                            # (4, og)
    mask = np.asarray(mask)
    shifts = np.asarray(shifts)

    in_maps = []
    for core in range(NCORES):
        sl = slice(core * NS, (core + 1) * NS)
        m = mask[sl].astype(np.float16)                          # (NS, O, I, K)
        m = m.transpose(1, 3, 2, 0)                              # (O, K, I, NS)
        m = m.reshape(NOG, 4, K, I, NS).transpose(0, 2, 1, 3, 4)
        maskg = np.ascontiguousarray(m.reshape(NOG, K, 128, NS))
        sh = shifts[sl].astype(np.int16)                         # (NS, K)
        idx = np.empty((128, K * 32), np.int16)
        for k in range(K):
            w = _wrap16(sh[:, k])                                # (16, 32)
            for a in range(8):
                idx[16 * a : 16 * (a + 1), 32 * k : 32 * (k + 1)] = w
        in_maps.append(
            {"xT4": xT4, "idx": idx, "maskg": maskg, "wt4": wt4, "bcol": bcol}
        )
    return in_maps


def kernel(x, Wconv, bconv, mask, shifts):
    nc = _build()
    in_maps = make_in_maps(x, Wconv, bconv, mask, shifts)
    res = run_bass_kernel_spmd(nc, in_maps, core_ids=list(range(NCORES)))
    y = np.empty((O, N), np.float32)
    for core in range(NCORES):
        y[:, core * NS : (core + 1) * NS] = res.results[core]["y"]
    return y
